# revision 1
# baseline (speedup 1.0000x reference)
"""Deformable-align kernel for 8 TRN2 NeuronCores (bedrock-safe).

Strategy: data-parallel, core i handles batch b=i//2, row-half r=i%2
(32 rows x 64 cols = 2048 pixels). No collectives.

Per-core pipeline (fp16 matmul operands, f32 accumulation):
  1. offset conv : 3x3 conv as 18 shifted matmuls into PSUM. Columns are
                   emitted in "q-order" q = p*16 + s where pixel
                   px = s*128 + p, so the per-pixel values land with
                   px%128 on partitions after a cheap fold-DMA.
  2. index/weight: floor via round-magic, g = clip(floor(v),0,62),
                   corner weights w_t = relu(1-|v-(g+t)|) (border-exact,
                   folds the reference's clamp+validity masking).
  3. gather      : host pre-builds x_quads[r] = x pixels [r,r+1,r+64,r+65]
                   (channels-last), so one indirect DMA per (tap, px-block)
                   with per-partition offsets idx = gy*64+gx fetches all 4
                   bilinear corners of 128 pixels (1024-elem runs).
  4. weighting   : pixels sit on partitions -> bilinear weights apply as
                   per-partition scalars (tensor_scalar/scalar_tensor_tensor
                   chains); all 4 corners summed on the fly.
  5. transpose   : TensorE transpose flips P[px,c] slabs into PSUM [c,px];
                   ACT copies them to SBUF as the main-matmul rhs.
  6. main matmul : out[o,px] accumulated over (c_lo,chi,k) = 18 matmuls per
                   (oc, px-chunk) + bias.
"""
import sys

import numpy as np

sys.path.insert(0, "/opt/trn_rl_repo")

import concourse.bass as bass
import concourse.tile as tile
from concourse import bacc, mybir
from concourse.bass_utils import run_bass_kernel_spmd

F16 = mybir.dt.float16
F32 = mybir.dt.float32
I32 = mybir.dt.int32
AF = mybir.ActivationFunctionType
OP = mybir.AluOpType

B, C, O, H, W = 4, 256, 256, 64, 64
K2 = 9
HALF = 32
PX = HALF * W            # 2048 per core
MAGIC = 12582912.0       # 1.5 * 2**23

_cache = {}


def build_nc():
    nc = bacc.Bacc(
        "TRN2", target_bir_lowering=False, debug=False,
        enable_asserts=False, num_devices=8,
    )
    xq = nc.dram_tensor("xq", [4096, 1024], F16, kind="ExternalInput")
    ypad = nc.dram_tensor("ypad", [128, 4488], F16, kind="ExternalInput")
    offw = nc.dram_tensor("offw", [128, 324], F16, kind="ExternalInput")
    dwt = nc.dram_tensor("dwt", [128, 4608], F16, kind="ExternalInput")
    offb = nc.dram_tensor("offb", [18, 1], F32, kind="ExternalInput")
    dbias = nc.dram_tensor("dbias", [128, 2], F32, kind="ExternalInput")
    basey = nc.dram_tensor("basey", [128, 144], F32, kind="ExternalInput")
    basex = nc.dram_tensor("basex", [128, 144], F32, kind="ExternalInput")
    ident = nc.dram_tensor("ident", [128, 128], F16, kind="ExternalInput")
    out = nc.dram_tensor("out", [256, 2048], F32, kind="ExternalOutput")

    with tile.TileContext(nc) as tc:
        build_body(tc, xq, ypad, offw, dwt, offb, dbias, basey, basex,
                   ident, out)
    nc.compile()
    return nc


def _sub(ap, off, dims):
    """Custom sub-AP of a tile AP: extra element offset + explicit free dims."""
    part = list(ap.ap[0])
    return bass.AP(ap.tensor, ap.offset + off, [part] + [list(d) for d in dims])


def build_body(tc, xq, ypad, offw, dwt, offb, dbias, basey, basex, ident, out):
    nc = tc.nc
    from contextlib import ExitStack

    with ExitStack() as ctx:
        cpool = ctx.enter_context(tc.tile_pool(name="consts", bufs=1))
        dw = cpool.tile([128, 4608], F16)
        nc.sync.dma_start(dw[:], dwt.ap())
        db = cpool.tile([128, 2], F32)
        nc.sync.dma_start(db[:], dbias.ap())
        idt = cpool.tile([128, 128], F16)
        nc.sync.dma_start(idt[:], ident.ap())

        ppool = ctx.enter_context(tc.tile_pool(name="persist", bufs=1))
        idxq = ppool.tile([128, 144], I32)       # quad row idx, [p,(k,b)]
        cw = ppool.tile([128, 576], F32)         # [p,(k, corner4, b)] wgts
        osb = [ppool.tile([128, 2048], F32, name=f"osb{oc}") for oc in range(2)]
        # main-mm rhs per tap: [c_lo, (chi, px)]
        pts = [ppool.tile([128, 4096], F16, name=f"pt{k}") for k in range(K2)]

        # ---------------- stage 1+2: offsets, indices, weights -------------
        with ExitStack() as s12:
            c1 = s12.enter_context(tc.tile_pool(name="c1", bufs=1))
            yp = c1.tile([128, 4488], F16)
            nc.sync.dma_start(yp[:], ypad.ap())
            ow = c1.tile([128, 324], F16)
            nc.sync.dma_start(ow[:], offw.ap())
            ob = c1.tile([18, 1], F32)
            nc.sync.dma_start(ob[:], offb.ap())
            by = c1.tile([128, 144], F32)
            nc.sync.dma_start(by[:], basey.ap())
            bx = c1.tile([128, 144], F32)
            nc.sync.dma_start(bx[:], basex.ap())

            ps1 = s12.enter_context(tc.tile_pool(name="ps1", bufs=1,
                                                 space="PSUM"))
            p_off = ps1.tile([18, 2048], F32)
            # columns in q-order: q = p*16+s, pixel px = s*128+p,
            # h_loc = s*2 + p1, w = p0  (p = p1*64+p0)
            for qc in range(4):          # N-chunks of 512: p in [32qc,32qc+32)
                p1, p0b = (32 * qc) // 64, (32 * qc) % 64
                n = 0
                for ki in range(3):
                    for kj in range(3):
                        for chi in range(2):
                            rhs = _sub(yp, chi * 2244 + (ki + p1) * 66
                                       + kj + p0b,
                                       [[1, 32], [132, 16]])
                            lhsT = _sub(ow, ((ki * 3 + kj) * 2 + chi) * 18,
                                        [[1, 18]])
                            nc.tensor.matmul(
                                p_off[:, qc * 512:(qc + 1) * 512], lhsT, rhs,
                                start=(n == 0), stop=(n == 17))
                            n += 1

            s2 = s12.enter_context(tc.tile_pool(name="s2", bufs=1))
            offs = s2.tile([18, 2048], F32)
            nc.vector.tensor_scalar(offs[:], p_off[:], ob[:, 0:1], None, OP.add)

            dyt = s2.tile([128, 144], F32)
            dxt = s2.tile([128, 144], F32)
            for k in range(K2):
                # fold q-order row into [p, s]: partition p <- 16-elem run
                nc.gpsimd.dma_start(
                    _sub(dyt, k * 16, [[1, 16]]),
                    offs[2 * k:2 * k + 1, :])
                nc.gpsimd.dma_start(
                    _sub(dxt, k * 16, [[1, 16]]),
                    offs[2 * k + 1:2 * k + 2, :])

            def corner(dt_, base, kind):
                v = s2.tile([128, 144], F32, name=f"v{kind}")
                nc.vector.tensor_add(v[:], dt_[:], base[:])
                t = s2.tile([128, 144], F32, name=f"t{kind}")
                nc.vector.tensor_scalar(t[:], v[:], -0.5, MAGIC, OP.add, OP.add)
                g = s2.tile([128, 144], F32, name=f"g{kind}")
                nc.vector.tensor_scalar(g[:], t[:], -MAGIC, 62.0, OP.add,
                                        OP.min)
                nc.vector.tensor_scalar(g[:], g[:], 0.0, None, OP.max)
                d = s2.tile([128, 144], F32, name=f"d{kind}")
                nc.vector.tensor_sub(d[:], v[:], g[:])
                a0 = s2.tile([128, 144], F32, name=f"a0{kind}")
                nc.scalar.activation(a0[:], d[:], AF.Abs)
                w0 = s2.tile([128, 144], F32, name=f"w0{kind}")
                nc.scalar.activation(w0[:], a0[:], AF.Relu, bias=1.0,
                                     scale=-1.0)
                a1 = s2.tile([128, 144], F32, name=f"a1{kind}")
                nc.scalar.activation(a1[:], d[:], AF.Abs, bias=1.0, scale=-1.0)
                w1 = s2.tile([128, 144], F32, name=f"w1{kind}")
                nc.scalar.activation(w1[:], a1[:], AF.Relu, bias=1.0,
                                     scale=-1.0)
                return g, (w0, w1)

            gy, wt = corner(dyt, by, "y")
            gx, ws = corner(dxt, bx, "x")

            idf = s2.tile([128, 144], F32)
            nc.vector.scalar_tensor_tensor(idf[:], gy[:], 64.0, gx[:],
                                           OP.mult, OP.add)
            nc.vector.tensor_copy(idxq[:], idf[:])

            # corner weights: corner order matches quad layout
            # [y0x0, y0x1, y1x0, y1x1]
            for i in range(2):
                for j in range(2):
                    dst = _sub(cw, (i * 2 + j) * 16, [[64, 9], [1, 16]])
                    nc.vector.tensor_mul(dst, wt[i][:], ws[j][:])

        # ---------------- stages 3-5: gather, weight, transpose ------------
        xq_ap = bass.AP(xq, 0, [[1024, 4096], [1, 1024]])

        with ExitStack() as s35:
            gpool = s35.enter_context(tc.tile_pool(name="gpool", bufs=6))
            wpool = s35.enter_context(tc.tile_pool(name="wpool", bufs=6))
            tps = s35.enter_context(tc.tile_pool(name="tpsum", bufs=4,
                                                 space="PSUM"))
            outps = s35.enter_context(tc.tile_pool(name="outpsum", bufs=2,
                                                   space="PSUM"))

            for k in range(K2):
                for b in range(16):
                    g = gpool.tile([128, 1024], F16, name="g", tag="g")
                    nc.gpsimd.indirect_dma_start(
                        out=g[:], out_offset=None, in_=xq_ap,
                        in_offset=bass.IndirectOffsetOnAxis(
                            ap=idxq[:, k * 16 + b:k * 16 + b + 1], axis=0))
                    # P_px[p, 256] = sum_corner cw * g[corner]
                    t1 = wpool.tile([128, 256], F16, name="t1", tag="t1")
                    nc.scalar.activation(
                        t1[:], g[:, 0:256], AF.Copy,
                        scale=cw[:, k * 64 + 0 * 16 + b:k * 64 + 0 * 16 + b + 1])
                    t2 = wpool.tile([128, 256], F16, name="t2", tag="t2")
                    nc.vector.scalar_tensor_tensor(
                        t2[:], g[:, 256:512],
                        cw[:, k * 64 + 1 * 16 + b:k * 64 + 1 * 16 + b + 1],
                        t1[:], OP.mult, OP.add)
                    t3 = wpool.tile([128, 256], F16, name="t3", tag="t3")
                    nc.vector.scalar_tensor_tensor(
                        t3[:], g[:, 512:768],
                        cw[:, k * 64 + 2 * 16 + b:k * 64 + 2 * 16 + b + 1],
                        t2[:], OP.mult, OP.add)
                    pp = wpool.tile([128, 256], F16, name="pp", tag="pp")
                    nc.vector.scalar_tensor_tensor(
                        pp[:], g[:, 768:1024],
                        cw[:, k * 64 + 3 * 16 + b:k * 64 + 3 * 16 + b + 1],
                        t3[:], OP.mult, OP.add)
                    for chi in range(2):
                        tp = tps.tile([128, 128], F16, name="tp", tag="tp")
                        nc.tensor.transpose(tp[:], pp[:, chi * 128:(chi + 1) * 128],
                                            idt[:])
                        nc.scalar.activation(
                            pts[k][:, chi * 2048 + b * 128:
                                   chi * 2048 + (b + 1) * 128],
                            tp[:], AF.Copy)

            # ---------------- stage 6: main matmul + bias ------------------
            for pxc in range(4):
                for oc in range(2):
                    po = outps.tile([128, 512], F32, name="po", tag="po")
                    n = 0
                    for k in range(K2):
                        for chi in range(2):
                            lhsT = _sub(dw, ((k * 2 + chi) * 2 + oc) * 128,
                                        [[1, 128]])
                            rhs = pts[k][:, chi * 2048 + pxc * 512:
                                         chi * 2048 + pxc * 512 + 512]
                            nc.tensor.matmul(po[:], lhsT, rhs,
                                             start=(n == 0), stop=(n == 17))
                            n += 1
                    nc.vector.tensor_scalar(
                        osb[oc][:, pxc * 512:(pxc + 1) * 512],
                        po[:], db[:, oc:oc + 1], None, OP.add)

            for oc in range(2):
                nc.sync.dma_start(out.ap()[oc * 128:(oc + 1) * 128, :],
                                  osb[oc][:])


def _prep_host(inputs):
    """Per-core input maps (host does layout only)."""
    x = np.asarray(inputs["x"], np.float32)
    y = np.asarray(inputs["y"], np.float32)
    offw = np.asarray(inputs["offset_w"], np.float32)
    offb = np.asarray(inputs["offset_b"], np.float32)
    dww = np.asarray(inputs["deform_w"], np.float32)
    dbb = np.asarray(inputs["deform_b"], np.float32)

    ow = np.zeros((128, 18, 18), np.float16)
    wr = offw.reshape(18, 2, 128, 3, 3)
    for k in range(9):
        for chi in range(2):
            ow[:, k * 2 + chi, :] = wr[:, chi, :, k // 3, k % 3].T
    dwm = np.zeros((128, 36, 128), np.float16)
    dr = dww.reshape(2, 128, 2, 128, 3, 3)
    for k in range(9):
        for chi in range(2):
            for oc in range(2):
                dwm[:, (k * 2 + chi) * 2 + oc, :] = \
                    dr[oc, :, chi, :, k // 3, k % 3].T

    dbias = dbb.reshape(2, 128).T.astype(np.float32).copy()
    offbt = offb.reshape(18, 1).astype(np.float32)
    ident = np.eye(128, dtype=np.float16)

    # base grids in [p, (k, s)] layout: pixel px = s*128 + p
    pv = np.arange(128)
    sv = np.arange(16)
    kiv = (np.arange(9) // 3).astype(np.float32)
    kjv = (np.arange(9) % 3).astype(np.float32)
    pxg = sv[None, :] * 128 + pv[:, None]           # [p, s]
    wg = (pxg % 64).astype(np.float32)
    hg = (pxg // 64).astype(np.float32)
    bx = np.zeros((128, 9, 16), np.float32)
    for k in range(9):
        bx[:, k, :] = wg + (kjv[k] - 1.0)

    in_maps = []
    for core in range(8):
        b, r = core // 2, core % 2
        xp = np.zeros((4097 + 65, 256), np.float32)
        xp[:4096] = x[b].transpose(1, 2, 0).reshape(4096, 256)
        quads = np.zeros((4096, 1024), np.float16)
        quads[:4031, 0:256] = xp[0:4031]
        quads[:4031, 256:512] = xp[1:4032]
        quads[:4031, 512:768] = xp[64:4095]
        quads[:4031, 768:1024] = xp[65:4096]

        yp = np.zeros((128, 2, 34, 66), np.float16)
        lo = r * HALF
        slo, shi = max(lo - 1, 0), min(lo + HALF + 1, H)
        ys = y[b, :, slo:shi, :].reshape(2, 128, shi - slo, W)
        yp[:, :, (slo - lo + 1):(shi - lo + 1), 1:65] = ys.transpose(1, 0, 2, 3)

        byw = np.zeros((128, 9, 16), np.float32)
        for k in range(9):
            byw[:, k, :] = r * HALF + hg + (kiv[k] - 1.0)

        in_maps.append({
            "xq": quads,
            "ypad": yp.reshape(128, 4488),
            "offw": ow.reshape(128, 324),
            "dwt": dwm.reshape(128, 4608),
            "offb": offbt,
            "dbias": dbias,
            "basey": byw.reshape(128, 144),
            "basex": bx.reshape(128, 144),
            "ident": ident,
        })
    return in_maps


def kernel(**inputs) -> np.ndarray:
    if "nc" not in _cache:
        _cache["nc"] = build_nc()
    nc = _cache["nc"]
    in_maps = _prep_host(inputs)
    res = run_bass_kernel_spmd(nc, in_maps, core_ids=list(range(8)))

    out = np.zeros((B, O, H, W), np.float32)
    for core in range(8):
        b, r = core // 2, core % 2
        o = res.results[core]["out"]          # [256, 2048], cols = flat px
        out[b, :, r * HALF:(r + 1) * HALF, :] = o.reshape(O, HALF, W)
    return out


if __name__ == "__main__":
    nc = build_nc()
    print("build OK")



# revision 64
# speedup vs baseline: 13456.9026x; 13456.9026x over previous
"""Deformable-align kernel for 8 TRN2 NeuronCores (bedrock-safe).

Strategy: data-parallel, core i handles batch b=i//2, row-half r=i%2
(32 rows x 64 cols = 2048 pixels). No collectives.

Per-core pipeline (fp16 matmul operands, f32 accumulation), processed in
two pixel halves h so the gather stream starts while half-1 offsets are
still being computed:
  1. offset conv : computed TRANSPOSED (pixels on partitions, 18 offset
                   channels as matmul columns) per 128-px chunk, so dy/dx
                   land in [p, (k,s)] layout via one strided psum->sbuf
                   copy — no DMA relayout. The conv bias is folded into
                   the host-built base grids.
  2. index/weight: floor via round-magic, g = clip(floor(v),0,62),
                   corner weights w_t = relu(1-|v-(g+t)|) (border-exact,
                   folds the reference's clamp+validity masking).
                   idx = gy*64+gx as int16, bounced through DRAM into
                   dma_gather's 16-partition wrap (replicated x8).
  3. gather      : host pre-builds x_quads[r] = x pixels [r,r+1,r+64,r+65]
                   (channels-last); ONE dma_gather per (tap, half) fetches
                   1024 quads (2KB each) = all 4 bilinear corners of 1024
                   pixels. 2x-sized SWDGE ring keeps 2 gathers in flight so
                   the DMA engines stream back-to-back (~5.8us/gather).
  4. weighting   : pixels sit on partitions -> per-corner scalings run as
                   independent ops spread over DVE (tensor_scalar, 4x
                   mode), ACT (activation*scale) and Pool; pair sums as
                   2x-mode tensor_tensor on DVE.
  5. transpose   : the two pair sums accumulate via PE transpose-matmuls
                   straight into PSUM (transpose is linear, so the final
                   corner add is free); ACT copies 512-wide slabs to SBUF.
  6. main matmul : progressive — each tap's 2 chi matmuls accumulate into
                   persistent [128,512] PSUM tiles right after its copies
                   land; bias + store fire on the last tap, leaving only a
                   few us of tail after the final gather.
"""
import sys

import numpy as np

sys.path.insert(0, "/opt/trn_rl_repo")

import concourse.bass as bass
import concourse.tile as tile
from concourse import bacc, mybir
from concourse.bass_utils import run_bass_kernel_spmd

F16 = mybir.dt.float16
F32 = mybir.dt.float32
I32 = mybir.dt.int32
I16 = mybir.dt.int16
AF = mybir.ActivationFunctionType
OP = mybir.AluOpType

B, C, O, H, W = 4, 256, 256, 64, 64
K2 = 9
HALF = 32
PX = HALF * W            # 2048 per core
MAGIC = 12582912.0       # 1.5 * 2**23

_cache = {}


DBG = []         # e.g. ["dyx", "idxq", "cw", "pts0"] -> extra outputs


def build_nc():
    nc = bacc.Bacc(
        "TRN2", target_bir_lowering=False, debug=False,
        enable_asserts=False, num_devices=8,
        # ring must hold 2 in-flight dma_gathers (1024+ descriptors each)
        dynamic_dma_scratch_size=65536,
    )
    xq = nc.dram_tensor("xq", [4096, 1024], F16, kind="ExternalInput")
    ypad = nc.dram_tensor("ypad", [128, 4488], F16, kind="ExternalInput")
    offw = nc.dram_tensor("offw", [128, 324], F16, kind="ExternalInput")
    dwt = nc.dram_tensor("dwt", [128, 4608], F16, kind="ExternalInput")
    dbias = nc.dram_tensor("dbias", [128, 2], F32, kind="ExternalInput")
    basey = nc.dram_tensor("basey", [128, 144], F32, kind="ExternalInput")
    basex = nc.dram_tensor("basex", [128, 144], F32, kind="ExternalInput")
    ident = nc.dram_tensor("ident", [128, 128], F16, kind="ExternalInput")
    out = nc.dram_tensor("out", [256, 2048], F32, kind="ExternalOutput")
    idxd = nc.dram_tensor("idxd", [16, 1152], I16, kind="Internal")
    dbg = {n: nc.dram_tensor(f"d_{n}",
                             [128, {"dyx": 288, "idxq": 144, "cw": 576,
                                    "pts0": 4096}[n]],
                             {"idxq": I16, "pts0": F16}.get(n, F32),
                             kind="ExternalOutput")
           for n in DBG}

    with tile.TileContext(nc) as tc:
        build_body(tc, xq, ypad, offw, dwt, dbias, basey, basex,
                   ident, out, idxd, dbg)
    nc.compile()
    return nc


def _sub(ap, off, dims):
    """Custom sub-AP of a tile AP: extra element offset + explicit free dims."""
    part = list(ap.ap[0])
    return bass.AP(ap.tensor, ap.offset + off, [part] + [list(d) for d in dims])


def build_body(tc, xq, ypad, offw, dwt, dbias, basey, basex, ident, out,
               idxd, dbg=None):
    dbg = dbg or {}
    nc = tc.nc
    from contextlib import ExitStack

    with ExitStack() as ctx:
        cpool = ctx.enter_context(tc.tile_pool(name="consts", bufs=1))
        ppool = ctx.enter_context(tc.tile_pool(name="persist", bufs=1))
        # [p, (h, k, s8)] quad row idx (int16 for dma_gather)
        idxq = ppool.tile([128, 144], I16)
        # dma_gather-wrapped indices: [r, (h, k, s8, q)], idx list element
        # i = s8*128 + q*16 + r at [r, col s8*8+q], replicated over the 8
        # 16-partition groups
        idxw = ppool.tile([128, 1152], I16)
        # [p, (h, corner4, k, s8)] corner weights
        cw = ppool.tile([128, 576], F32)
        # main-mm rhs per tap: [c_lo, (chi, px)]
        pts = [ppool.tile([128, 4096], F16, name=f"pt{k}") for k in range(K2)]

        gpool = ctx.enter_context(tc.tile_pool(name="gpool", bufs=3))
        xq_ap = bass.AP(xq, 0, [[1024, 4096], [1, 1024]])
        gt = {}
        units = [(h, k) for h in range(2) for k in range(K2)]

        def issue(u):
            h, k = units[u]
            g = gpool.tile([128, 8192], F16, name="g", tag="g")
            c0 = h * 576 + k * 64
            if u == len(units) - 1:
                # split the final gather so its weighting (the kernel's tail)
                # starts half a transfer earlier
                for hf in range(2):
                    nc.gpsimd.dma_gather(
                        out_ap=_sub(g[:], hf * 4096, [[1024, 4], [1, 1024]]),
                        in_ap=xq_ap, idxs_ap=idxw[:, c0 + 32 * hf:
                                                  c0 + 32 * hf + 32],
                        num_idxs=512, num_idxs_reg=512, elem_size=1024)
            else:
                nc.gpsimd.dma_gather(
                    out_ap=_sub(g[:], 0, [[1024, 8], [1, 1024]]), in_ap=xq_ap,
                    idxs_ap=idxw[:, c0:c0 + 64],
                    num_idxs=1024, num_idxs_reg=1024, elem_size=1024)
            gt[u] = g

        def wrap_idx(h):
            # bounce idxq through DRAM into dma_gather's 16-partition wrap:
            # idxd[r, h*576 + k*64 + s8*8 + q] = idxq[q*16+r, h*72+k*8+s8],
            # then load back replicated into all 8 partition groups.
            # Split per q / group to keep every AP 3-dim (balancer limit);
            # alternate HWDGE & Pool-SWDGE queues to halve serialization.
            for q in range(8):
                dst = bass.AP(idxd, h * 576 + q, [[1152, 16], [8, 72]])
                src = idxq[q * 16:(q + 1) * 16, h * 72:(h + 1) * 72]
                # h0: split HWDGE/Pool (prologue latency); h1: HWDGE only so
                # Pool's gather stream is never delayed
                (nc.gpsimd if h == 0 and q < 4 else nc.sync).dma_start(dst,
                                                                       src)
            dst = idxw[:, h * 576:(h + 1) * 576]
            src = bass.AP(idxd, h * 576, [[0, 8], [1152, 16], [1, 576]])
            nc.sync.dma_start(dst, src)

        # ---------------- stage 1+2: offsets, indices, weights -------------
        # processed per pixel-half h so the first gathers can issue while the
        # second half's offsets are still being computed
        if True:
            c1 = ctx.enter_context(tc.tile_pool(name="c1", bufs=1))
            cscope = ExitStack()
            cvpool = cscope.enter_context(
                tc.tile_pool(name="cvps", bufs=2, space="PSUM"))
            s2 = ctx.enter_context(tc.tile_pool(name="s2", bufs=1))
            # dy/dx in [p, (k, dyx2, s)] layout, filled per 128-px chunk by a
            # strided psum->sbuf copy (no DMA relayout needed: the offset
            # conv is computed transposed, pixels on partitions)
            dyx = s2.tile([128, 288], F32)

            yscope = ExitStack()
            ypool = yscope.enter_context(tc.tile_pool(name="ypool", bufs=1))
            yp = ypool.tile([128, 4488], F16)
            nc.sync.dma_start(yp[:], ypad.ap())
            ow = ypool.tile([128, 324], F16)
            nc.sync.dma_start(ow[:], offw.ap())
            by = c1.tile([128, 144], F32)
            nc.sync.dma_start(by[:], basey.ap())
            bx = c1.tile([128, 144], F32)
            nc.sync.dma_start(bx[:], basex.ap())
            # bulky consts only needed later: separate queue + late priority
            # so they don't delay the stage-1 inputs on the DMA device
            dw = cpool.tile([128, 4608], F16)
            db = cpool.tile([128, 2], F32)
            idt = cpool.tile([128, 128], F16)
            with tc.high_priority(offset=-500):
                nc.scalar.dma_start(idt[:], ident.ap())
                nc.scalar.dma_start(dw[:], dwt.ap())
                nc.scalar.dma_start(db[:], dbias.ap())

            def stage1(h):
                # chunk s holds pixels s*128..s*128+127: partition p is the
                # in-chunk pixel (row 2s + p//64, col p%64). Double-buffered
                # psum tiles so chunk s+1's matmuls overlap chunk s's copy.
                for s in range(8 * h, 8 * h + 8):
                    cps = cvpool.tile([128, 18], F32, name="cps", tag="cps")
                    for p1 in range(2):     # image row 2s + p1 -> partitions
                        n = 0               # p1*64 .. p1*64+63
                        for ki in range(3):
                            for kj in range(3):
                                for chi in range(2):
                                    lhsT = _sub(yp, chi * 2244
                                                + (2 * s + p1 + ki) * 66 + kj,
                                                [[1, 64]])
                                    rhs = _sub(ow,
                                               ((ki * 3 + kj) * 2 + chi) * 18,
                                               [[1, 18]])
                                    nc.tensor.matmul(
                                        cps[p1 * 64:(p1 + 1) * 64, :],
                                        lhsT, rhs, start=(n == 0),
                                        stop=(n == 17))
                                    n += 1
                    # [128, (k, dy/dx)] strided copy into dyx columns
                    dst = _sub(dyx, s, [[32, 9], [16, 2]])
                    src = _sub(cps[:], 0, [[2, 9], [1, 2]])
                    if s % 2 == 0:
                        nc.vector.tensor_copy(dst, src)
                    else:
                        nc.scalar.activation(dst, src, AF.Copy)

            def stage2(h):
                def corner(dof, base, kind):
                    dsl = _sub(dyx, dof + h * 8, [[32, 9], [1, 8]])
                    bsl = _sub(base, h * 8, [[16, 9], [1, 8]])
                    v = s2.tile([128, 72], F32, name=f"v{kind}")
                    nc.vector.tensor_add(v[:], dsl, bsl)
                    t = s2.tile([128, 72], F32, name=f"t{kind}")
                    nc.vector.tensor_scalar(t[:], v[:], -0.5, MAGIC, OP.add,
                                            OP.add)
                    g = s2.tile([128, 72], F32, name=f"g{kind}")
                    nc.vector.tensor_scalar(g[:], t[:], -MAGIC, 62.0, OP.add,
                                            OP.min)
                    nc.vector.tensor_scalar(g[:], g[:], 0.0, None, OP.max)
                    d = s2.tile([128, 72], F32, name=f"d{kind}")
                    nc.vector.tensor_sub(d[:], v[:], g[:])
                    a0 = s2.tile([128, 72], F32, name=f"a0{kind}")
                    nc.scalar.activation(a0[:], d[:], AF.Abs)
                    w0 = s2.tile([128, 72], F32, name=f"w0{kind}")
                    nc.scalar.activation(w0[:], a0[:], AF.Relu, bias=1.0,
                                         scale=-1.0)
                    a1 = s2.tile([128, 72], F32, name=f"a1{kind}")
                    nc.scalar.activation(a1[:], d[:], AF.Abs, bias=1.0,
                                         scale=-1.0)
                    w1 = s2.tile([128, 72], F32, name=f"w1{kind}")
                    nc.scalar.activation(w1[:], a1[:], AF.Relu, bias=1.0,
                                         scale=-1.0)
                    return g, (w0, w1)

                gy, wt = corner(0, by, "y")
                gx, ws = corner(16, bx, "x")

                idf = s2.tile([128, 72], F32, name="idf")
                nc.vector.scalar_tensor_tensor(idf[:], gy[:], 64.0, gx[:],
                                               OP.mult, OP.add)
                nc.vector.tensor_copy(idxq[:, h * 72:(h + 1) * 72], idf[:])
                # corner order matches quad layout [y0x0, y0x1, y1x0, y1x1]
                for i in range(2):
                    for j in range(2):
                        dst = _sub(cw, h * 288 + (i * 2 + j) * 72, [[1, 72]])
                        nc.vector.tensor_mul(dst, wt[i][:], ws[j][:])

            stage1(0)
            stage2(0)
            wrap_idx(0)
            issue(0)
            issue(1)
            stage1(1)
            yscope.close()      # free yp/ow for the unit-loop pools
            cscope.close()      # free the conv psum bank for the po tiles

        # ---------------- stages 3-6: gather, weight, transpose, matmul ----
        with ExitStack() as s35:
            wpool = s35.enter_context(tc.tile_pool(name="wpool", bufs=4))
            opool = s35.enter_context(tc.tile_pool(name="opool", bufs=2))
            tps = s35.enter_context(tc.tile_pool(name="tpsum", bufs=2,
                                                 space="PSUM"))
            outps = s35.enter_context(tc.tile_pool(name="outpsum", bufs=4,
                                                   space="PSUM"))

            po = {}

            def mm_partial(pxc, oc, k):
                # accumulate tap k's contribution into the persistent psum
                # tile for (pxc, oc); bias+store on the last tap
                if k == 0:
                    po[(pxc, oc)] = outps.tile([128, 512], F32, name="po",
                                               tag="po")
                t = po[(pxc, oc)]
                for chi in range(2):
                    lhsT = _sub(dw, ((k * 2 + chi) * 2 + oc) * 128,
                                [[1, 128]])
                    rhs = pts[k][:, chi * 2048 + pxc * 512:
                                 chi * 2048 + pxc * 512 + 512]
                    nc.tensor.matmul(t[:], lhsT, rhs,
                                     start=(k == 0 and chi == 0),
                                     stop=(k == K2 - 1 and chi == 1))
                if k == K2 - 1:
                    ot = opool.tile([128, 512], F32, name="ot", tag="ot")
                    nc.vector.tensor_scalar(ot[:], t[:], db[:, oc:oc + 1],
                                            None, OP.add)
                    nc.sync.dma_start(
                        out.ap()[oc * 128:(oc + 1) * 128,
                                 pxc * 512:(pxc + 1) * 512],
                        ot[:])

            # software pipeline: keep 2 gathers in flight ahead of compute so
            # the DMA engines stream back-to-back; interleave main-matmul
            # pieces once their pixel range is complete
            for u in range(len(units)):
                if u + 2 < len(units):
                    issue(u + 2)
                h, k = units[u]
                g = gt.pop(u)
                if True:
                    tp = None
                    for bl in range(8):
                        b = h * 8 + bl
                        gof = bl * 1024
                        cwc = lambda c: cw[:, h * 288 + c * 72 + k * 8 + bl:
                                           h * 288 + c * 72 + k * 8 + bl + 1]
                        # independent per-corner scalings spread over engines
                        m0 = wpool.tile([128, 256], F16, name="m0", tag="m0")
                        nc.vector.tensor_scalar(
                            m0[:], g[:, gof:gof + 256], cwc(0), None, OP.mult)
                        m1 = wpool.tile([128, 256], F16, name="m1", tag="m1")
                        nc.vector.tensor_scalar(
                            m1[:], g[:, gof + 256:gof + 512], cwc(1), None,
                            OP.mult)
                        m2 = wpool.tile([128, 256], F16, name="m2", tag="m2")
                        nc.scalar.activation(
                            m2[:], g[:, gof + 512:gof + 768], AF.Copy,
                            scale=cwc(2))
                        m3 = wpool.tile([128, 256], F16, name="m3", tag="m3")
                        nc.gpsimd.tensor_scalar(
                            m3[:], g[:, gof + 768:gof + 1024], cwc(3), None,
                            OP.mult)
                        s01 = wpool.tile([128, 256], F16, name="s01",
                                         tag="s01")
                        nc.vector.tensor_tensor(s01[:], m0[:], m1[:], OP.add)
                        s23 = wpool.tile([128, 256], F16, name="s23",
                                         tag="s23")
                        nc.vector.tensor_tensor(s23[:], m2[:], m3[:], OP.add)
                        # transpose-accumulate the two pair sums on PE:
                        # tp[c, px] += s01^T + s23^T  (4 blocks per psum tile)
                        if bl % 4 == 0:
                            tp = [tps.tile([128, 512], F32, name=f"tp{chi}",
                                           tag=f"tp{chi}") for chi in range(2)]
                        sl = (bl % 4) * 128
                        for chi in range(2):
                            dst = tp[chi][:, sl:sl + 128]
                            nc.tensor.matmul(
                                dst, s01[:, chi * 128:(chi + 1) * 128], idt[:],
                                start=True, stop=False)
                            nc.tensor.matmul(
                                dst, s23[:, chi * 128:(chi + 1) * 128], idt[:],
                                start=False, stop=True)
                        if bl % 4 == 3:
                            bq = h * 2 + bl // 4
                            for chi in range(2):
                                nc.scalar.activation(
                                    pts[k][:, chi * 2048 + bq * 512:
                                           chi * 2048 + (bq + 1) * 512],
                                    tp[chi][:], AF.Copy)
                            pxc = 2 * h + bl // 4
                            for oc in range(2):
                                mm_partial(pxc, oc, k)
                if u == len(units) - 1:
                    for n, t in (("dyx", dyx), ("idxq", idxq), ("cw", cw),
                                 ("pts0", pts[0])):
                        if n in dbg:
                            nc.sync.dma_start(dbg[n].ap(), t[:])
                if u == 3:
                    # deferred: h1's index/weight chain (inputs ready early,
                    # outputs first needed by unit 9's gather ~50us later)
                    stage2(1)
                    wrap_idx(1)


def _prep_host(inputs):
    """Per-core input maps (host does layout only)."""
    x = np.asarray(inputs["x"], np.float32)
    y = np.asarray(inputs["y"], np.float32)
    offw = np.asarray(inputs["offset_w"], np.float32)
    offb = np.asarray(inputs["offset_b"], np.float32)
    dww = np.asarray(inputs["deform_w"], np.float32)
    dbb = np.asarray(inputs["deform_b"], np.float32)

    ow = np.zeros((128, 18, 18), np.float16)
    wr = offw.reshape(18, 2, 128, 3, 3)
    for k in range(9):
        for chi in range(2):
            ow[:, k * 2 + chi, :] = wr[:, chi, :, k // 3, k % 3].T
    dwm = np.zeros((128, 36, 128), np.float16)
    dr = dww.reshape(2, 128, 2, 128, 3, 3)
    for k in range(9):
        for chi in range(2):
            for oc in range(2):
                dwm[:, (k * 2 + chi) * 2 + oc, :] = \
                    dr[oc, :, chi, :, k // 3, k % 3].T

    dbias = dbb.reshape(2, 128).T.astype(np.float32).copy()
    ident = np.eye(128, dtype=np.float16)

    # base grids in [p, (k, s)] layout, offset-conv bias folded in:
    # pixel px = s*128 + p
    pv = np.arange(128)
    sv = np.arange(16)
    kiv = (np.arange(9) // 3).astype(np.float32)
    kjv = (np.arange(9) % 3).astype(np.float32)
    pxg = sv[None, :] * 128 + pv[:, None]           # [p, s]
    wg = (pxg % 64).astype(np.float32)
    hg = (pxg // 64).astype(np.float32)
    bx = np.zeros((128, 9, 16), np.float32)
    for k in range(9):
        bx[:, k, :] = wg + (kjv[k] - 1.0) + offb[2 * k + 1]

    in_maps = []
    for core in range(8):
        b, r = core // 2, core % 2
        xp = np.zeros((4097 + 65, 256), np.float32)
        xp[:4096] = x[b].transpose(1, 2, 0).reshape(4096, 256)
        quads = np.zeros((4096, 1024), np.float16)
        quads[:4031, 0:256] = xp[0:4031]
        quads[:4031, 256:512] = xp[1:4032]
        quads[:4031, 512:768] = xp[64:4095]
        quads[:4031, 768:1024] = xp[65:4096]

        yp = np.zeros((128, 2, 34, 66), np.float16)
        lo = r * HALF
        slo, shi = max(lo - 1, 0), min(lo + HALF + 1, H)
        ys = y[b, :, slo:shi, :].reshape(2, 128, shi - slo, W)
        yp[:, :, (slo - lo + 1):(shi - lo + 1), 1:65] = ys.transpose(1, 0, 2, 3)

        byw = np.zeros((128, 9, 16), np.float32)
        for k in range(9):
            byw[:, k, :] = r * HALF + hg + (kiv[k] - 1.0) + offb[2 * k]

        in_maps.append({
            "xq": quads,
            "ypad": yp.reshape(128, 4488),
            "offw": ow.reshape(128, 324),
            "dwt": dwm.reshape(128, 4608),
            "dbias": dbias,
            "basey": byw.reshape(128, 144),
            "basex": bx.reshape(128, 144),
            "ident": ident,
        })
    return in_maps


def kernel(**inputs) -> np.ndarray:
    if "nc" not in _cache:
        _cache["nc"] = build_nc()
    nc = _cache["nc"]
    in_maps = _prep_host(inputs)
    res = run_bass_kernel_spmd(nc, in_maps, core_ids=list(range(8)))

    out = np.zeros((B, O, H, W), np.float32)
    for core in range(8):
        b, r = core // 2, core % 2
        o = res.results[core]["out"]          # [256, 2048], cols = flat px
        out[b, :, r * HALF:(r + 1) * HALF, :] = o.reshape(O, HALF, W)
    return out


if __name__ == "__main__":
    nc = build_nc()
    print("build OK")

